# revision 1
# baseline (speedup 1.0000x reference)
"""Trainium2 Bass kernel for nn_NERModel loss (CE + quadruplet + context MSE).

Strategy (8 NeuronCores, data-parallel over batch):
  - Each core processes 8 batches = 8192 tokens of embeddings [8192, 384] f32.
  - Tokens are tiled 128/tile at stride 127 (65 tiles) so every adjacent-token
    pair falls inside some tile; host-built per-(tile,slot) weights de-dup
    overlapping tokens/pairs exactly once.
  - CE: PE transposes emb chunks (PSUM), ScE/VE copy to SBUF, then
    logitsT[17,512] = W.T-chunk (lhsT) @ embT (rhs) accumulated over 3 K-chunks.
    exp on ScE with per-partition bias=b (free bias add), per-token sel via a
    fused tensor_tensor_reduce against a host-built ce_w-scaled one-hot, and
    sumexp column sums via an accumulating row-placement matmul into one
    persistent PSUM bank. One ln at the end.
  - CTX: one matmul per tile with constant (S - I) weights produces adjacent
    diffs straight into PSUM; fused square+pair-weight+row-reduce split across
    ScE (activation Square, scale=w, accum_out) and VE (tensor_tensor_reduce).
  - Device returns two partial sums per core; host does the tiny quadruplet
    term (index scans over labels + 49 gathered rows) and final combination.
"""

import sys

for _p in ("/opt/trn_rl_repo", "/root/.axon_site/_ro/trn_rl_repo"):
    if _p not in sys.path:
        sys.path.append(_p)

import numpy as np
from contextlib import ExitStack

import concourse.bass as bass
import concourse.bacc as bacc
import concourse.mybir as mybir
from concourse import tile
from concourse.ap import AP

NUM_LABELS = 17
MARGIN = 1.0
IGNORE = -100

B, S, H, L = 64, 1024, 384, NUM_LABELS
NCORES = 8
BP = B // NCORES            # batches per core
NTOK = BP * S               # tokens per core (8192)
STRIDE = 127                # token stride between tiles (1-token overlap)
NT = 65                     # tiles per core
NG = (NT + 3) // 4          # compute groups of 4 tiles -> 17
GDMA = 8                    # tiles per DMA transfer
NDMA = (NT + GDMA - 1) // GDMA  # 9
F32 = mybir.dt.float32


def _tile_start(t: int) -> int:
    # last tile is clamped so it stays in-bounds; duplicated tokens/pairs are
    # zero-weighted on the host side
    return NTOK - 128 if t == NT - 1 else STRIDE * t


def _build_nc() -> bass.Bass:
    import os

    skip_ctx = bool(os.environ.get("NER_SKIP_CTX"))
    skip_ce = bool(os.environ.get("NER_SKIP_CE"))
    no_gpsimd = bool(os.environ.get("NER_NO_GPSIMD_MEMSET"))
    # Bacc (not plain Bass): its compile() legalizes sync waits (>=2 waits per
    # instruction are split / moved to LDWEIGHTS), which walrus requires.
    nc = bacc.Bacc("TRN2", debug=False)

    emb = nc.declare_dram_parameter("emb", [NTOK, H], F32, isOutput=False)
    woh = nc.declare_dram_parameter("woh", [L, NG * 512], F32, isOutput=False)
    cewg = nc.declare_dram_parameter("cewg", [NG, 512], F32, isOutput=False)
    pairw = nc.declare_dram_parameter("pairw", [128, NT], F32, isOutput=False)
    wt = nc.declare_dram_parameter("wt", [128, 3 * L], F32, isOutput=False)
    bcol = nc.declare_dram_parameter("bcol", [L, 1], F32, isOutput=False)
    selg = nc.declare_dram_parameter("selg", [L, NG * L], F32, isOutput=False)
    dfw = nc.declare_dram_parameter("dfw", [128, 128], F32, isOutput=False)
    idn = nc.declare_dram_parameter("idn", [128, 128], F32, isOutput=False)
    ones = nc.declare_dram_parameter("ones", [128, 1], F32, isOutput=False)
    outv = nc.declare_dram_parameter("outv", [1, 8], F32, isOutput=True)

    AF = mybir.ActivationFunctionType
    AX = mybir.AxisListType
    OP = mybir.AluOpType

    with tile.TileContext(nc) as tc, ExitStack() as ctx:
        consts = ctx.enter_context(tc.tile_pool(name="consts", bufs=1))
        nat_pool = ctx.enter_context(tc.tile_pool(name="nat", bufs=3))
        embt_pool = ctx.enter_context(tc.tile_pool(name="embt", bufs=2))
        expt_pool = ctx.enter_context(tc.tile_pool(name="expt", bufs=2))
        junk_pool = ctx.enter_context(tc.tile_pool(name="junk", bufs=2))
        acc_pool = ctx.enter_context(tc.tile_pool(name="acc", bufs=1))
        ps_t = ctx.enter_context(tc.tile_pool(name="ps_t", bufs=1, space="PSUM"))
        ps_l = ctx.enter_context(tc.tile_pool(name="ps_l", bufs=1, space="PSUM"))
        ps_d = ctx.enter_context(tc.tile_pool(name="ps_d", bufs=1, space="PSUM"))
        ps_s = ctx.enter_context(tc.tile_pool(name="ps_s", bufs=1, space="PSUM"))

        def cload(handle, shape):
            t = consts.tile(list(shape), F32, tag=handle.name + "_c")
            nc.sync.dma_start(out=t[:], in_=handle.ap())
            return t

        woh_t = cload(woh, (L, NG * 512))
        cewg_t = cload(cewg, (NG, 512))
        pairw_t = cload(pairw, (128, NT))
        wt_t = cload(wt, (128, 3 * L))
        bcol_t = cload(bcol, (L, 1))
        selg_t = cload(selg, (L, NG * L))
        dfw_t = cload(dfw, (128, 128))
        idn_t = cload(idn, (128, 128))
        ones_t = cload(ones, (128, 1))

        # persistent accumulators
        sumexp_ps = ps_s.tile([L, 512], F32)          # [group, group-token]
        ctxbuf = acc_pool.tile([128, NT], F32)        # per-tile weighted ||diff||^2
        selbuf = acc_pool.tile([L, NG], F32)          # per-group sum cew*logit
        nc.vector.memset(selbuf[:], 0.0)

        nat_tiles = {}

        simple_dma = bool(os.environ.get("NER_SIMPLE_DMA"))
        skip_emb_dma = bool(os.environ.get("NER_SKIP_EMB_DMA"))

        def do_dma(d: int):
            ntl = min(GDMA, NT - d * GDMA)
            nat = nat_pool.tile([128, GDMA * H], F32, tag="natbuf")
            if skip_emb_dma:
                nat_tiles[d] = nat
                return
            if simple_dma:
                for j in range(ntl):
                    src = AP(
                        tensor=emb,
                        offset=_tile_start(d * GDMA + j) * H,
                        ap=[[H, 128], [1, H]],
                    )
                    nc.sync.dma_start(out=nat[:, j * H : (j + 1) * H], in_=src)
            elif ntl == GDMA:
                src = AP(
                    tensor=emb,
                    offset=_tile_start(d * GDMA) * H,
                    ap=[[H, 128], [STRIDE * H, GDMA], [1, H]],
                )
                nc.sync.dma_start(out=nat[:, :].rearrange("p (g h) -> p g h", h=H), in_=src)
            else:
                src = AP(
                    tensor=emb,
                    offset=_tile_start(d * GDMA) * H,
                    ap=[[H, 128], [1, H]],
                )
                nc.sync.dma_start(out=nat[:, 0:H], in_=src)
            nat_tiles[d] = nat

        def nat_slice(t: int, c0: int, c1: int):
            nat = nat_tiles[t // GDMA]
            base = (t % GDMA) * H
            return nat[:, base + c0 : base + c1]

        def _ctx_only(tiles):
            for half in range(2):
                tiles_h = tiles[2 * half : 2 * half + 2]
                if not tiles_h:
                    break
                df_ps = ps_d.tile([128, 2, 512], F32, tag="df_ps")
                for jj, t in enumerate(tiles_h):
                    nc.tensor.matmul(
                        df_ps[:, jj, 0:H], dfw_t[:], nat_slice(t, 0, H),
                        start=True, stop=True,
                    )
                _sqw(tiles_h, df_ps)

        def do_group(g: int):
            tiles = list(range(4 * g, min(4 * g + 4, NT)))
            last = len(tiles) < 4

            # ---- transposes: embT[h, tok] chunks ----
            if skip_ce:
                _ctx_only(tiles)
                return
            embT_ps = ps_t.tile([128, 3 * 512], F32, tag="embT_ps")
            for j, t in enumerate(tiles):
                for c in range(3):
                    # out = nat_chunk.T via a normal matmul against identity
                    # (transpose-mode LW has too few sync-wait slots in codegen)
                    nc.tensor.matmul(
                        embT_ps[:, c * 512 + j * 128 : c * 512 + (j + 1) * 128],
                        nat_slice(t, c * 128, (c + 1) * 128),
                        idn_t[:],
                        start=True,
                        stop=True,
                    )
            embT = embt_pool.tile([128, 3 * 512], F32, tag="embT")
            if last:
                # only j=0 columns are real; zero the rest so downstream
                # full-width ops read finite garbage
                (nc.vector if no_gpsimd else nc.gpsimd).memset(embT[:], 0.0)
                ev = embT[:, :].rearrange("p (c k) -> p c k", k=512)
                pv = embT_ps[:, :].rearrange("p (c k) -> p c k", k=512)
                nc.vector.tensor_copy(ev[:, :, 0:128], pv[:, :, 0:128])
            else:
                nc.vector.tensor_copy(embT[:], embT_ps[:])

            # ---- logitsT [17, 512] ----
            lg_ps = ps_l.tile([L, 512], F32, tag="lg_ps")
            for c in range(3):
                nc.tensor.matmul(
                    lg_ps[:],
                    wt_t[:, c * L : (c + 1) * L],
                    embT[:, c * 512 : (c + 1) * 512],
                    start=(c == 0),
                    stop=(c == 2),
                )

            # ---- exp(logit + b) ----
            expT = expt_pool.tile([L, 512], F32, tag="expT")
            nc.scalar.activation(expT[:], lg_ps[:], AF.Exp, bias=bcol_t[:, 0:1], scale=1.0)

            # ---- sel accumulation: selacc += sum(logit * woh) ----
            junk17 = junk_pool.tile([L, 512], F32, tag="junk17")
            nc.vector.tensor_mul(junk17[:], lg_ps[:], woh_t[:, g * 512 : (g + 1) * 512])
            junk17c = junk_pool.tile([L, 512], F32, tag="junk17b")
            nc.vector.tensor_scalar(
                out=junk17c[:], in0=junk17[:], scalar1=1.0, scalar2=None,
                op0=OP.mult, op1=OP.add, accum_out=selbuf[:, g : g + 1],
            )

            # ---- sumexp row-placement matmul ----
            nc.tensor.matmul(
                sumexp_ps[:],
                selg_t[:, g * L : (g + 1) * L],
                expT[:],
                start=(g == 0),
                stop=(g == NG - 1),
            )

            # ---- ctx: diff = emb[t+1]-emb[t] via (S-I) matmul, then w*||diff||^2 ----
            # each matmul output must live inside one 512-col PSUM bank, so
            # pad each tile's diff region to 512 and process 2 tiles per alloc
            if skip_ctx:
                return
            for half in range(2):
                tiles_h = tiles[2 * half : 2 * half + 2]
                if not tiles_h:
                    break
                df_ps = ps_d.tile([128, 2, 512], F32, tag="df_ps")
                for jj, t in enumerate(tiles_h):
                    nc.tensor.matmul(
                        df_ps[:, jj, 0:H],
                        dfw_t[:],
                        nat_slice(t, 0, H),
                        start=True,
                        stop=True,
                    )
                _sqw(tiles_h, df_ps)

        def _sqw(tiles_h, df_ps):
            for jj, t in enumerate(tiles_h):
                dsl = df_ps[:, jj, 0:H]
                if False:
                    pass
                else:
                    jk = junk_pool.tile([128, H], F32, tag="junkS")
                    nc.scalar.activation(
                        jk[:],
                        dsl,
                        AF.Square,
                        bias=0.0,
                        scale=pairw_t[:, t : t + 1],
                        accum_out=ctxbuf[:, t : t + 1],
                    )

        g_done = 0
        for d in range(NDMA):
            do_dma(d)
            # run all compute groups fully covered by the DMAs issued so far
            tiles_ready = min((d + 1) * GDMA, NT)
            while g_done < NG and min(4 * g_done + 4, NT) <= tiles_ready:
                do_group(g_done)
                g_done += 1
        assert g_done == NG

        # ---- final reduction ----
        skip_final = bool(os.environ.get("NER_SKIP_FINAL"))
        if skip_final:
            outs0 = acc_pool.tile([1, 8], F32)
            nc.vector.memset(outs0[:], 0.0)
            nc.sync.dma_start(out=outv.ap(), in_=outs0[:])
        if skip_ce:
            nc.vector.memset(sumexp_ps[:], 1.0)
        if skip_ctx:
            nc.vector.memset(ctxbuf[:], 0.0)
        if not skip_final:
            lnsum = expt_pool.tile([L, 512], F32, tag="lnsum")
            nc.scalar.activation(lnsum[:], sumexp_ps[:], AF.Ln)
            accA = acc_pool.tile([L, 1], F32)
            junk17b = junk_pool.tile([L, 512], F32, tag="junk17")
            nc.vector.tensor_mul(junk17b[:], lnsum[:], cewg_t[:])
            junk17d = junk_pool.tile([L, 512], F32, tag="junk17b")
            nc.vector.tensor_scalar(
                out=junk17d[:], in0=junk17b[:], scalar1=1.0, scalar2=None,
                op0=OP.mult, op1=OP.add, accum_out=accA[:, 0:1],
            )
            selacc = acc_pool.tile([L, 1], F32)
            junkS = junk_pool.tile([L, NG], F32, tag="junkS17")
            nc.vector.tensor_scalar(
                out=junkS[:], in0=selbuf[:], scalar1=1.0, scalar2=None,
                op0=OP.mult, op1=OP.add, accum_out=selacc[:, 0:1],
            )
            cev = acc_pool.tile([L, 1], F32)
            nc.vector.tensor_sub(cev[:], accA[:], selacc[:])
            fin1 = ps_l.tile([1, 1], F32, tag="lg_ps")
            nc.tensor.matmul(fin1[:], cev[:], ones_t[0:L, :], start=True, stop=True)

            ctxsum = acc_pool.tile([128, 1], F32)
            nc.vector.tensor_reduce(ctxsum[:], ctxbuf[:], axis=AX.X, op=OP.add)
            fin2 = ps_l.tile([1, 1], F32, tag="lg_ps")
            nc.tensor.matmul(fin2[:], ctxsum[:], ones_t[:], start=True, stop=True)

            outs = acc_pool.tile([1, 8], F32)
            nc.vector.memset(outs[:], 0.0)
            nc.scalar.copy(outs[0:1, 0:1], fin1[:])
            nc.scalar.copy(outs[0:1, 1:2], fin2[:])
            nc.sync.dma_start(out=outv.ap(), in_=outs[:])

    nc.compile()
    return nc


# ---------------------------------------------------------------------------
# host-side preparation


def _host_grids(labf: np.ndarray, mskf: np.ndarray):
    """Per-core grids. labf/mskf: [NTOK] int64/int32.

    Returns (cew_grid [NT,128], pairw_grid [NT,128], woh [L, NG*512],
             cewg [NG, 512])."""
    valid = labf != IGNORE
    pair_ok = np.zeros(NTOK, dtype=bool)
    lf = labf.astype(np.int64)
    # pair (k, k+1) within a batch row of length S
    k = np.arange(NTOK - 1)
    in_batch = (k % S) != (S - 1)
    pair_ok[:-1] = in_batch & (lf[:-1] != IGNORE) & (lf[:-1] == lf[1:]) & (lf[:-1] > 0)

    cew_grid = np.zeros((NT, 128), np.float32)
    pairw_grid = np.zeros((NT, 128), np.float32)
    seen_tok = np.zeros(NTOK, dtype=bool)
    seen_pair = np.zeros(NTOK, dtype=bool)
    tokmap = np.zeros((NT, 128), np.int64)
    for t in range(NT):
        s0 = _tile_start(t)
        toks = np.arange(s0, s0 + 128)
        tokmap[t] = toks
        fresh = ~seen_tok[toks]
        cew_grid[t] = (valid[toks] & fresh).astype(np.float32)
        seen_tok[toks] = True
        pfresh = ~seen_pair[toks]
        pw = pair_ok[toks] & pfresh
        pw[127] = False  # col 127 diff is garbage by construction
        pairw_grid[t] = pw.astype(np.float32)
        seen_pair[toks[:127]] = True

    woh = np.zeros((L, NG * 512), np.float32)
    cewg = np.zeros((NG, 512), np.float32)
    for g in range(NG):
        for j in range(min(4, NT - 4 * g)):
            t = 4 * g + j
            toks = tokmap[t]
            cols = g * 512 + j * 128 + np.arange(128)
            cewg[g, j * 128 : (j + 1) * 128] = cew_grid[t]
            lab_c = np.where(valid[toks], lf[toks], 0)
            woh[lab_c, cols] = cew_grid[t]
    return cew_grid, pairw_grid, woh, cewg


def _quad_host(fe: np.ndarray, fl: np.ndarray, fm: np.ndarray) -> np.float32:
    """Mirror of the reference quadruplet loss in numpy float32."""
    N = fe.shape[0]
    idx = np.arange(N, dtype=np.int64)
    BIG = N
    fm_b = fm > 0
    is_ent = fm_b & (fl > 0)
    non_ent = fm_b & (fl == 0)
    d_i = np.min(np.where(non_ent, idx, BIG))
    has_non = bool(non_ent.any())

    a_i = np.zeros(L - 1, np.int64)
    p_i = np.zeros(L - 1, np.int64)
    n_i = np.zeros(L - 1, np.int64)
    ok = np.zeros(L - 1, bool)
    for i, t in enumerate(range(1, L)):
        m = is_ent & (fl == t)
        order = np.sort(np.where(m, idx, BIG))
        a_i[i], p_i[i] = order[0], order[1]
        cnt = int(m.sum())
        other = is_ent & (fl != t)
        n_i[i] = np.min(np.where(other, idx, BIG))
        ok[i] = (cnt >= 2) and bool(other.any()) and has_non

    clip = lambda v: np.clip(v, 0, N - 1)
    A = fe[clip(a_i)]
    P = fe[clip(p_i)]
    Ng = fe[clip(n_i)]
    D = fe[clip(np.array([d_i]))]
    eps = np.float32(1e-6)

    def dist(x, y):
        d = (x - y + eps).astype(np.float32)
        return np.sqrt(np.sum(d * d, axis=-1, dtype=np.float32)).astype(np.float32)

    pd, nd, dd = dist(A, P), dist(A, Ng), dist(A, D)
    ql = np.maximum(pd - nd + np.float32(MARGIN), 0) + np.maximum(
        pd - dd + np.float32(2.0 * MARGIN), 0
    )
    qcnt = int(ok.sum())
    quad = float(np.sum(np.where(ok, ql, 0.0), dtype=np.float64)) / max(qcnt, 1)
    return np.float32(quad if qcnt > 0 else 0.0)


_NC_CACHE = {}


def _get_nc():
    if "nc" not in _NC_CACHE:
        _NC_CACHE["nc"] = _build_nc()
    return _NC_CACHE["nc"]


def _device_consts():
    if "consts" in _NC_CACHE:
        return _NC_CACHE["consts"]
    dfw = np.zeros((128, 128), np.float32)
    for t in range(127):
        dfw[t + 1, t] = 1.0
    dfw[np.arange(128), np.arange(128)] -= 1.0
    idn = np.eye(128, dtype=np.float32)
    ones = np.ones((128, 1), np.float32)
    selg = np.zeros((L, NG * L), np.float32)
    for g in range(NG):
        selg[:, g * L + g] = 1.0
    _NC_CACHE["consts"] = (dfw, idn, ones, selg)
    return _NC_CACHE["consts"]


def kernel(embeddings, classifier_w, classifier_b, labels, attention_mask):
    from concourse.bass_utils import run_bass_kernel_spmd

    emb = np.ascontiguousarray(np.asarray(embeddings, dtype=np.float32))
    W = np.asarray(classifier_w, dtype=np.float32)
    b = np.asarray(classifier_b, dtype=np.float32)
    lab = np.asarray(labels)
    msk = np.asarray(attention_mask)

    lab_f = lab.reshape(-1).astype(np.int64)
    msk_f = msk.reshape(-1).astype(np.int64)
    N = B * S

    wt = np.zeros((128, 3 * L), np.float32)
    for c in range(3):
        wt[:, c * L : (c + 1) * L] = W[:, c * 128 : (c + 1) * 128].T
    bcol = b.reshape(L, 1).astype(np.float32)
    dfw, idn, ones, selg = _device_consts()

    in_maps = []
    cew_grids = []
    for cidx in range(NCORES):
        sl = slice(cidx * NTOK, (cidx + 1) * NTOK)
        labc = lab_f[sl]
        cewg_grid, pairw_grid, woh, cewg = _host_grids(labc, msk_f[sl])
        cew_grids.append(cewg_grid)
        in_maps.append(
            {
                "emb": emb.reshape(N, H)[sl],
                "woh": woh,
                "cewg": cewg,
                "pairw": np.ascontiguousarray(pairw_grid.T),
                "wt": wt,
                "bcol": bcol,
                "selg": selg,
                "dfw": dfw,
                "idn": idn,
                "ones": ones,
            }
        )

    nc = _get_nc()
    res = run_bass_kernel_spmd(nc, in_maps, list(range(NCORES)))

    ce_sum = 0.0
    ctx_sum = 0.0
    for cidx in range(NCORES):
        out = res.results[cidx]["outv"]
        ce_sum += float(out[0, 0])
        ctx_sum += float(out[0, 1])

    valid = lab_f != IGNORE
    ce_cnt = int(valid.sum())
    # device sel used logits without bias; correct with sum(cew * b[label])
    lab_safe = np.where(valid, lab_f, 0)
    ce_sum -= float(np.sum(np.where(valid, b[lab_safe], 0.0), dtype=np.float64))
    ce = ce_sum / max(ce_cnt, 1)

    pair_ok = np.zeros(N, dtype=bool)
    k = np.arange(N - 1)
    in_batch = (k % S) != (S - 1)
    pair_ok[:-1] = (
        in_batch & (lab_f[:-1] != IGNORE) & (lab_f[:-1] == lab_f[1:]) & (lab_f[:-1] > 0)
    )
    pc = int(pair_ok.sum())
    ctx = (ctx_sum / H) / max(pc, 1) if pc > 0 else 0.0

    quad = _quad_host(emb.reshape(N, H), lab_f, msk_f)

    loss = ce + 0.5 * float(quad) + 0.1 * ctx
    return np.float32(loss)



# revision 7
# speedup vs baseline: 1.8505x; 1.8505x over previous
"""Trainium2 Bass kernel for nn_NERModel loss (CE + quadruplet + context MSE).

Strategy (8 NeuronCores, data-parallel over batch):
  - Host casts embeddings to bf16; each core streams 8192 tokens
    [8192, 384] bf16 (6.3 MB) instead of 12.6 MB f32.
  - Tokens tiled 128/tile at stride 127 (65 tiles) so every adjacent
    pair falls inside some tile; host-built weights de-dup overlaps.
  - All matmuls run in bf16 (1 cyc/row on PE vs 4 for f32):
      * 12 identity-matmul transposes per group of 4 tiles -> embT
        chunks in PSUM, copied to SBUF bf16 (copies round-robined over
        Scalar/GpSimd engines to keep DVE free),
      * logitsT[17,512] = wtT-chunk @ embT-chunk over 3 K-chunks,
      * sumexp via a row-placement matmul accumulating all 17 groups
        into one persistent PSUM bank,
      * ctx: adjacent-token diffs are computed in the transposed layout
        on DVE (bf16 4x mode), squared on DVE, then per-pair sums via a
        ones-column matmul accumulated into a second persistent bank.
  - Device returns CE and CTX partial sums; host does the tiny
    quadruplet term and the final combination in f64/np.
"""

import sys

for _p in ("/opt/trn_rl_repo", "/root/.axon_site/_ro/trn_rl_repo"):
    if _p not in sys.path:
        sys.path.append(_p)

import numpy as np
from contextlib import ExitStack

import ml_dtypes

import concourse.bass as bass
import concourse.bacc as bacc
import concourse.mybir as mybir
from concourse import tile
from concourse.ap import AP

NUM_LABELS = 17
MARGIN = 1.0
IGNORE = -100

B, S, H, L = 64, 1024, 384, NUM_LABELS
NCORES = 8
BP = B // NCORES            # batches per core
NTOK = BP * S               # tokens per core (8192)
STRIDE = 127                # token stride between tiles (1-token overlap)
NT = 65                     # tiles per core
NG = (NT + 3) // 4          # compute groups of 4 tiles -> 17
GDMA = 8                    # tiles per DMA transfer
NDMA = (NT + GDMA - 1) // GDMA  # 9
F32 = mybir.dt.float32
BF16 = mybir.dt.bfloat16
BF16_NP = ml_dtypes.bfloat16


def _tile_start(t: int) -> int:
    # last tile is clamped so it stays in-bounds; duplicated tokens/pairs
    # are zero-weighted on the host side
    return NTOK - 128 if t == NT - 1 else STRIDE * t


def _build_nc() -> bass.Bass:
    import os

    skip_ctx = bool(os.environ.get("NER_SKIP_CTX"))
    skip_sel = bool(os.environ.get("NER_SKIP_SEL"))
    skip_exp = bool(os.environ.get("NER_SKIP_EXP"))
    nc = bacc.Bacc("TRN2", debug=False)

    emb = nc.declare_dram_parameter("emb", [NTOK, H], BF16, isOutput=False)
    woh = nc.declare_dram_parameter("woh", [L, NG * 512], F32, isOutput=False)
    cewg = nc.declare_dram_parameter("cewg", [NG, 512], F32, isOutput=False)
    pairw = nc.declare_dram_parameter("pairw", [NG, 512], F32, isOutput=False)
    wt = nc.declare_dram_parameter("wt", [128, 3 * L], BF16, isOutput=False)
    bcol = nc.declare_dram_parameter("bcol", [L, 1], F32, isOutput=False)
    selg = nc.declare_dram_parameter("selg", [L, NG * L], BF16, isOutput=False)
    oneg = nc.declare_dram_parameter("oneg", [128, NG * L], BF16, isOutput=False)
    idn = nc.declare_dram_parameter("idn", [128, 128], BF16, isOutput=False)
    ones = nc.declare_dram_parameter("ones", [128, 1], F32, isOutput=False)
    outv = nc.declare_dram_parameter("outv", [1, 8], F32, isOutput=True)

    AF = mybir.ActivationFunctionType
    AX = mybir.AxisListType
    OP = mybir.AluOpType

    with tile.TileContext(nc) as tc, ExitStack() as ctx:
        consts = ctx.enter_context(tc.tile_pool(name="consts", bufs=1))
        nat_pool = ctx.enter_context(tc.tile_pool(name="nat", bufs=4))
        embt_pool = ctx.enter_context(tc.tile_pool(name="embt", bufs=2))
        sqd_pool = ctx.enter_context(tc.tile_pool(name="sqd", bufs=2))
        expt_pool = ctx.enter_context(tc.tile_pool(name="expt", bufs=2))
        junk_pool = ctx.enter_context(tc.tile_pool(name="junk", bufs=2))
        acc_pool = ctx.enter_context(tc.tile_pool(name="acc", bufs=1))
        ps_t = ctx.enter_context(tc.tile_pool(name="ps_t", bufs=4, space="PSUM"))
        ps_l = ctx.enter_context(tc.tile_pool(name="ps_l", bufs=2, space="PSUM"))
        ps_s = ctx.enter_context(tc.tile_pool(name="ps_s", bufs=1, space="PSUM"))
        ps_c = ctx.enter_context(tc.tile_pool(name="ps_c", bufs=1, space="PSUM"))

        def cload(handle, shape, dt):
            t = consts.tile(list(shape), dt, tag=handle.name + "_c")
            nc.sync.dma_start(out=t[:], in_=handle.ap())
            return t

        woh_t = cload(woh, (L, NG * 512), F32)
        cewg_t = cload(cewg, (NG, 512), F32)
        pairw_t = cload(pairw, (NG, 512), F32)
        wt_t = cload(wt, (128, 3 * L), BF16)
        bcol_t = cload(bcol, (L, 1), F32)
        selg_t = cload(selg, (L, NG * L), BF16)
        oneg_t = cload(oneg, (128, NG * L), BF16)
        idn_t = cload(idn, (128, 128), BF16)
        ones_t = cload(ones, (128, 1), F32)

        # persistent accumulators
        sumexp_ps = ps_s.tile([L, 512], F32)          # [group, group-token]
        ctx_ps = ps_c.tile([L, 512], F32)             # [group, group-pair]
        selbuf = acc_pool.tile([1, NG], F32)          # per-group sum w*logit
        nc.gpsimd.memset(selbuf[:], 0.0)

        nat_tiles = {}

        def do_dma(d: int):
            ntl = min(GDMA, NT - d * GDMA)
            nat = nat_pool.tile([128, GDMA * H], BF16, tag="natbuf")
            if ntl == GDMA:
                src = AP(
                    tensor=emb,
                    offset=_tile_start(d * GDMA) * H,
                    ap=[[H, 128], [STRIDE * H, GDMA], [1, H]],
                )
                nc.sync.dma_start(
                    out=nat[:, :].rearrange("p (g h) -> p g h", h=H), in_=src
                )
            else:
                src = AP(
                    tensor=emb,
                    offset=_tile_start(d * GDMA) * H,
                    ap=[[H, 128], [1, H]],
                )
                nc.sync.dma_start(out=nat[:, 0:H], in_=src)
            nat_tiles[d] = nat

        def nat_slice(t: int, c0: int, c1: int):
            nat = nat_tiles[t // GDMA]
            base = (t % GDMA) * H
            return nat[:, base + c0 : base + c1]

        def do_group(g: int):
            tiles = list(range(4 * g, min(4 * g + 4, NT)))
            last = len(tiles) < 4

            # ---- transposes: embT[h, tok] chunks (bf16 identity matmul) ----
            embT = embt_pool.tile([128, 3 * 512], BF16, tag="embT")
            if last:
                # only j=0 columns are real; zero the rest so downstream
                # full-width ops read exact zeros
                nc.gpsimd.memset(embT[:], 0.0)
            for c in range(3):
                embT_ps = ps_t.tile([128, 512], F32, tag="embT_ps")
                for j, t in enumerate(tiles):
                    nc.tensor.matmul(
                        embT_ps[:, j * 128 : (j + 1) * 128],
                        nat_slice(t, c * 128, (c + 1) * 128),
                        idn_t[:],
                        start=True,
                        stop=True,
                    )
                dst = embT[:, c * 512 : c * 512 + 128 * len(tiles)]
                src = embT_ps[:, 0 : 128 * len(tiles)]
                # GpSimd cannot read PSUM, so the cast copies go 2x Scalar
                # + 1x DVE (DVE also carries diffs/squares/sel)
                if c == 1:
                    nc.vector.tensor_copy(dst, src)
                else:
                    nc.scalar.copy(dst, src)

            # ---- logitsT [17, 512] ----
            lg_ps = ps_l.tile([L, 512], F32, tag="lg_ps")
            for c in range(3):
                nc.tensor.matmul(
                    lg_ps[:],
                    wt_t[:, c * L : (c + 1) * L],
                    embT[:, c * 512 : (c + 1) * 512],
                    start=(c == 0),
                    stop=(c == 2),
                )

            # ---- exp(logit + b) -> bf16 ----
            if not skip_exp:
                expT = expt_pool.tile([L, 512], BF16, tag="expT")
                nc.scalar.activation(expT[:], lg_ps[:], AF.Exp, bias=bcol_t[:, 0:1], scale=1.0)

            # ---- sel accumulation: selbuf[:, g] = sum_tok(logit * woh) ----
            if not skip_sel:
                junk17 = junk_pool.tile([L, 512], F32, tag="junk17")
                nc.vector.tensor_tensor(
                    out=junk17[:],
                    in0=lg_ps[:],
                    in1=woh_t[:, g * 512 : (g + 1) * 512],
                    op=OP.mult,
                )
                nc.gpsimd.tensor_reduce(
                    out=selbuf[0:1, g : g + 1], in_=junk17[:],
                    axis=AX.XYZWC, op=OP.add,
                )

            # ---- sumexp row-placement matmul ----
            if not skip_exp:
                nc.tensor.matmul(
                    sumexp_ps[:],
                    selg_t[:, g * L : (g + 1) * L],
                    expT[:],
                    start=(g == 0),
                    stop=(g == NG - 1),
                )

            # ---- ctx: adjacent-token diffs in transposed layout (DVE 4x) ----
            if skip_ctx:
                return
            dT = sqd_pool.tile([128, 1535], BF16, tag="dT")
            nc.vector.tensor_tensor(
                out=dT[:], in0=embT[:, 1:1536], in1=embT[:, 0:1535],
                op=OP.subtract,
            )
            sqdT = sqd_pool.tile([128, 1535], BF16, tag="sqdT")
            nc.vector.tensor_tensor(
                out=sqdT[:], in0=dT[:], in1=dT[:], op=OP.mult,
            )
            for c in range(3):
                nc.tensor.matmul(
                    ctx_ps[:, 0:511],
                    oneg_t[:, g * L : (g + 1) * L],
                    sqdT[:, c * 512 : c * 512 + 511],
                    start=(g == 0 and c == 0),
                    stop=(g == NG - 1 and c == 2),
                )

        g_done = 0
        for d in range(NDMA):
            do_dma(d)
            tiles_ready = min((d + 1) * GDMA, NT)
            while g_done < NG and min(4 * g_done + 4, NT) <= tiles_ready:
                do_group(g_done)
                g_done += 1
        assert g_done == NG

        # ---- final reduction ----
        if skip_exp:
            nc.vector.memset(sumexp_ps[:], 1.0)
        if skip_ctx:
            nc.vector.memset(ctx_ps[:], 0.0)
        lnsum = junk_pool.tile([L, 512], F32, tag="lnsum")
        nc.scalar.activation(lnsum[:], sumexp_ps[:], AF.Ln)
        accA = acc_pool.tile([L, 1], F32)
        junkA = junk_pool.tile([L, 512], F32, tag="junk17")
        nc.vector.tensor_tensor(out=junkA[:], in0=lnsum[:], in1=cewg_t[:], op=OP.mult)
        junkA2 = junk_pool.tile([L, 512], F32, tag="junk17b")
        nc.vector.tensor_scalar(
            out=junkA2[:], in0=junkA[:], scalar1=1.0, scalar2=None,
            op0=OP.mult, op1=OP.add, accum_out=accA[:, 0:1],
        )
        selacc = acc_pool.tile([1, 1], F32)
        junkS = junk_pool.tile([1, NG], F32, tag="junkS17")
        nc.vector.tensor_scalar(
            out=junkS[:], in0=selbuf[:], scalar1=1.0, scalar2=None,
            op0=OP.mult, op1=OP.add, accum_out=selacc[:, 0:1],
        )
        fin1 = ps_l.tile([1, 1], F32, tag="lg_ps")
        nc.tensor.matmul(fin1[:], accA[:], ones_t[0:L, :], start=True, stop=True)

        # ctx: weight the per-pair sums and total them
        ctxacc = acc_pool.tile([L, 1], F32)
        junkC = junk_pool.tile([L, 511], F32, tag="junkC")
        nc.vector.tensor_tensor(
            out=junkC[:], in0=ctx_ps[:, 0:511], in1=pairw_t[:, 0:511], op=OP.mult,
        )
        junkC2 = junk_pool.tile([L, 511], F32, tag="junkC2")
        nc.vector.tensor_scalar(
            out=junkC2[:], in0=junkC[:], scalar1=1.0, scalar2=None,
            op0=OP.mult, op1=OP.add, accum_out=ctxacc[:, 0:1],
        )
        fin2 = ps_l.tile([1, 1], F32, tag="lg_ps")
        nc.tensor.matmul(fin2[:], ctxacc[:], ones_t[0:L, :], start=True, stop=True)

        outs = acc_pool.tile([1, 8], F32)
        nc.vector.memset(outs[:], 0.0)
        nc.scalar.copy(outs[0:1, 0:1], fin1[:])
        nc.scalar.copy(outs[0:1, 1:2], fin2[:])
        nc.vector.tensor_copy(outs[0:1, 2:3], selacc[:])
        nc.sync.dma_start(out=outv.ap(), in_=outs[:])

    nc.compile()
    return nc


# ---------------------------------------------------------------------------
# host-side preparation


def _host_grids(labf: np.ndarray, mskf: np.ndarray):
    """Per-core grids. labf/mskf: [NTOK] int64/int32.

    Returns (woh [L, NG*512], cewg [NG, 512], pairw [NG, 512])."""
    valid = labf != IGNORE
    lf = labf.astype(np.int64)
    pair_ok = np.zeros(NTOK, dtype=bool)
    k = np.arange(NTOK - 1)
    in_batch = (k % S) != (S - 1)
    pair_ok[:-1] = in_batch & (lf[:-1] != IGNORE) & (lf[:-1] == lf[1:]) & (lf[:-1] > 0)

    woh = np.zeros((L, NG * 512), np.float32)
    cewg = np.zeros((NG, 512), np.float32)
    pairw = np.zeros((NG, 512), np.float32)
    seen_tok = np.zeros(NTOK, dtype=bool)
    seen_pair = np.zeros(NTOK, dtype=bool)
    for g in range(NG):
        ntl = min(4, NT - 4 * g)
        toks = np.zeros(512, np.int64)
        real = np.zeros(512, bool)
        for j in range(ntl):
            s0 = _tile_start(4 * g + j)
            toks[j * 128 : (j + 1) * 128] = np.arange(s0, s0 + 128)
            real[j * 128 : (j + 1) * 128] = True
        rt = toks[real]
        fresh = ~seen_tok[rt]
        cw = np.zeros(512, np.float32)
        cw[real] = (valid[rt] & fresh).astype(np.float32)
        seen_tok[rt] = True
        cewg[g] = cw
        lab_c = np.where(valid[toks] & real, lf[toks], 0)
        woh[lab_c, g * 512 + np.arange(512)] = cw

        # pairs: embT column j vs j+1; cols 127/255/383 are duplicates
        # (diff == 0), cross-tile and cross-group pairs covered by overlap
        for j in range(min(511, ntl * 128 - 1)):
            if j % 128 == 127:
                continue
            t0 = toks[j]
            if pair_ok[t0] and not seen_pair[t0]:
                pairw[g, j] = 1.0
                seen_pair[t0] = True
    return woh, cewg, pairw


def _quad_host(fe: np.ndarray, fl: np.ndarray, fm: np.ndarray) -> np.float32:
    """Mirror of the reference quadruplet loss in numpy float32."""
    N = fe.shape[0]
    idx = np.arange(N, dtype=np.int64)
    BIG = N
    fm_b = fm > 0
    is_ent = fm_b & (fl > 0)
    non_ent = fm_b & (fl == 0)
    d_i = np.min(np.where(non_ent, idx, BIG))
    has_non = bool(non_ent.any())

    a_i = np.zeros(L - 1, np.int64)
    p_i = np.zeros(L - 1, np.int64)
    n_i = np.zeros(L - 1, np.int64)
    ok = np.zeros(L - 1, bool)
    for i, t in enumerate(range(1, L)):
        m = is_ent & (fl == t)
        order = np.sort(np.where(m, idx, BIG))
        a_i[i], p_i[i] = order[0], order[1]
        cnt = int(m.sum())
        other = is_ent & (fl != t)
        n_i[i] = np.min(np.where(other, idx, BIG))
        ok[i] = (cnt >= 2) and bool(other.any()) and has_non

    clip = lambda v: np.clip(v, 0, N - 1)
    A = fe[clip(a_i)]
    P = fe[clip(p_i)]
    Ng = fe[clip(n_i)]
    D = fe[clip(np.array([d_i]))]
    eps = np.float32(1e-6)

    def dist(x, y):
        d = (x - y + eps).astype(np.float32)
        return np.sqrt(np.sum(d * d, axis=-1, dtype=np.float32)).astype(np.float32)

    pd, nd, dd = dist(A, P), dist(A, Ng), dist(A, D)
    ql = np.maximum(pd - nd + np.float32(MARGIN), 0) + np.maximum(
        pd - dd + np.float32(2.0 * MARGIN), 0
    )
    qcnt = int(ok.sum())
    quad = float(np.sum(np.where(ok, ql, 0.0), dtype=np.float64)) / max(qcnt, 1)
    return np.float32(quad if qcnt > 0 else 0.0)


_NC_CACHE = {}


def _get_nc():
    if "nc" not in _NC_CACHE:
        _NC_CACHE["nc"] = _build_nc()
    return _NC_CACHE["nc"]


def _device_consts():
    if "consts" in _NC_CACHE:
        return _NC_CACHE["consts"]
    idn = np.eye(128, dtype=BF16_NP)
    ones = np.ones((128, 1), np.float32)
    selg = np.zeros((L, NG * L), BF16_NP)
    oneg = np.zeros((128, NG * L), BF16_NP)
    for g in range(NG):
        selg[:, g * L + g] = 1.0
        oneg[:, g * L + g] = 1.0
    _NC_CACHE["consts"] = (idn, ones, selg, oneg)
    return _NC_CACHE["consts"]


def build_in_maps(embeddings, classifier_w, classifier_b, labels, attention_mask):
    emb = np.ascontiguousarray(np.asarray(embeddings, dtype=np.float32))
    W = np.asarray(classifier_w, dtype=np.float32)
    b = np.asarray(classifier_b, dtype=np.float32)
    lab_f = np.asarray(labels).reshape(-1).astype(np.int64)
    msk_f = np.asarray(attention_mask).reshape(-1).astype(np.int64)
    N = B * S

    emb_bf = emb.reshape(N, H).astype(BF16_NP)
    wt = np.zeros((128, 3 * L), BF16_NP)
    for c in range(3):
        wt[:, c * L : (c + 1) * L] = W[:, c * 128 : (c + 1) * 128].T.astype(BF16_NP)
    bcol = b.reshape(L, 1).astype(np.float32)
    idn, ones, selg, oneg = _device_consts()

    in_maps = []
    for cidx in range(NCORES):
        sl = slice(cidx * NTOK, (cidx + 1) * NTOK)
        woh, cewg, pairw = _host_grids(lab_f[sl], msk_f[sl])
        in_maps.append(
            {
                "emb": emb_bf[sl],
                "woh": woh,
                "cewg": cewg,
                "pairw": pairw,
                "wt": wt,
                "bcol": bcol,
                "selg": selg,
                "oneg": oneg,
                "idn": idn,
                "ones": ones,
            }
        )
    return in_maps, emb, lab_f, msk_f, b


def kernel(embeddings, classifier_w, classifier_b, labels, attention_mask):
    from concourse.bass_utils import run_bass_kernel_spmd

    in_maps, emb, lab_f, msk_f, b = build_in_maps(
        embeddings, classifier_w, classifier_b, labels, attention_mask
    )
    N = B * S

    nc = _get_nc()
    res = run_bass_kernel_spmd(nc, in_maps, list(range(NCORES)))

    ce_sum = 0.0
    ctx_sum = 0.0
    for cidx in range(NCORES):
        out = res.results[cidx]["outv"]
        ce_sum += float(out[0, 0]) - float(out[0, 2])
        ctx_sum += float(out[0, 1])

    valid = lab_f != IGNORE
    ce_cnt = int(valid.sum())
    # device sel used logits without bias; correct with sum(cew * b[label])
    lab_safe = np.where(valid, lab_f, 0)
    ce_sum -= float(np.sum(np.where(valid, b[lab_safe], 0.0), dtype=np.float64))
    ce = ce_sum / max(ce_cnt, 1)

    pair_ok = np.zeros(N, dtype=bool)
    k = np.arange(N - 1)
    in_batch = (k % S) != (S - 1)
    pair_ok[:-1] = (
        in_batch & (lab_f[:-1] != IGNORE) & (lab_f[:-1] == lab_f[1:]) & (lab_f[:-1] > 0)
    )
    pc = int(pair_ok.sum())
    ctx = (ctx_sum / H) / max(pc, 1) if pc > 0 else 0.0

    quad = _quad_host(emb.reshape(N, H), lab_f, msk_f)

    loss = ce + 0.5 * float(quad) + 0.1 * ctx
    return np.float32(loss)


# revision 8
# speedup vs baseline: 2.5589x; 1.3828x over previous
"""Trainium2 Bass kernel for nn_NERModel loss (CE + quadruplet + context MSE).

v4 strategy (8 NeuronCores, data-parallel over batch):
  - Host pre-transposes each core's embeddings to bf16 embT [384, 8192]
    (h-major): no on-device transposes, no PSUM round-trips, and the DMA
    moves 6.3 MB/core as 16 KB-contiguous descriptors at full rate.
  - Tokens stream in 4 quarters of 2048 columns; 16 CE groups of 512.
  - PE (all bf16): logitsT[17,512] per group over 3 K-chunks; per-token
    sumexp via a row-placement matmul into one persistent PSUM bank;
    ctx per-pair sums via ones-column matmuls into a second bank.
  - DVE: adjacent-column diffs + squares per quarter slab; sel partial
    (logit * one-hot weight) per group, accumulated on ScE.
  - Device returns CE-lse / sel / ctx partials; host adds the tiny
    quadruplet term (49 gathered rows) and combines.
"""

import sys

for _p in ("/opt/trn_rl_repo", "/root/.axon_site/_ro/trn_rl_repo"):
    if _p not in sys.path:
        sys.path.append(_p)

import numpy as np
from contextlib import ExitStack

import ml_dtypes

import concourse.bass as bass
import concourse.bacc as bacc
import concourse.mybir as mybir
from concourse import tile
from concourse.ap import AP

NUM_LABELS = 17
MARGIN = 1.0
IGNORE = -100

B, S, H, L = 64, 1024, 384, NUM_LABELS
NCORES = 8
BP = B // NCORES            # batches per core
NTOK = BP * S               # tokens per core (8192)
NG = 16                     # CE groups of 512 tokens
NQ = 4                      # DMA quarters of 2048 columns
QW = NTOK // NQ             # 2048
F32 = mybir.dt.float32
BF16 = mybir.dt.bfloat16
BF16_NP = ml_dtypes.bfloat16


def _build_nc() -> bass.Bass:
    nc = bacc.Bacc("TRN2", debug=False)

    embt = nc.declare_dram_parameter("embt", [H, NTOK], BF16, isOutput=False)
    woh = nc.declare_dram_parameter("woh", [L, NTOK], BF16, isOutput=False)
    cewg = nc.declare_dram_parameter("cewg", [NG, 512], F32, isOutput=False)
    pairw = nc.declare_dram_parameter("pairw", [NG, 512], F32, isOutput=False)
    wt = nc.declare_dram_parameter("wt", [128, 3 * L], BF16, isOutput=False)
    bcol = nc.declare_dram_parameter("bcol", [L, 1], F32, isOutput=False)
    outv = nc.declare_dram_parameter("outv", [1, 8], F32, isOutput=True)

    AF = mybir.ActivationFunctionType
    AX = mybir.AxisListType
    OP = mybir.AluOpType

    with tile.TileContext(nc) as tc, ExitStack() as ctx:
        consts = ctx.enter_context(tc.tile_pool(name="consts", bufs=1))
        big = ctx.enter_context(tc.tile_pool(name="big", bufs=1))
        sqd_pool = ctx.enter_context(tc.tile_pool(name="sqd", bufs=2))
        expt_pool = ctx.enter_context(tc.tile_pool(name="expt", bufs=2))
        junk_pool = ctx.enter_context(tc.tile_pool(name="junk", bufs=3))
        acc_pool = ctx.enter_context(tc.tile_pool(name="acc", bufs=1))
        ps_l = ctx.enter_context(tc.tile_pool(name="ps_l", bufs=3, space="PSUM"))
        ps_s = ctx.enter_context(tc.tile_pool(name="ps_s", bufs=1, space="PSUM"))
        ps_c = ctx.enter_context(tc.tile_pool(name="ps_c", bufs=1, space="PSUM"))

        def cload(handle, shape, dt):
            t = consts.tile(list(shape), dt, tag=handle.name + "_c")
            nc.sync.dma_start(out=t[:], in_=handle.ap())
            return t

        wt_t = cload(wt, (128, 3 * L), BF16)
        bcol_t = cload(bcol, (L, 1), F32)
        cewg_t = cload(cewg, (NG, 512), F32)
        pairw_t = cload(pairw, (NG, 512), F32)

        # device-built structured consts (DMA of tiny bf16 mats lowers to
        # per-element descriptors and stalls the sync queue for ~20us)
        selg_t = consts.tile([L, NG * NG], BF16, tag="selg")
        oneg_t = consts.tile([128, NG * NG], BF16, tag="oneg")
        ones_t = consts.tile([128, 1], F32, tag="ones")
        nc.gpsimd.memset(selg_t[:], 0.0)
        nc.gpsimd.memset(oneg_t[:], 0.0)
        nc.gpsimd.memset(ones_t[:], 1.0)
        for g in range(NG):
            nc.gpsimd.memset(selg_t[:, g * NG + g : g * NG + g + 1], 1.0)
            nc.gpsimd.memset(oneg_t[:, g * NG + g : g * NG + g + 1], 1.0)

        # embT resident: [128, 3, NTOK] (chunk c = h rows 128c..128c+127)
        embT = big.tile([128, 3 * NTOK], BF16, tag="embT")
        ev = embT[:, :].rearrange("p (c k) -> p c k", k=NTOK)

        # persistent accumulators
        sumexp_ps = ps_s.tile([NG, 512], F32)         # [group, token-in-group]
        ctx_ps = ps_c.tile([NG, 512], F32)            # [group, pair-in-group]
        selbuf = acc_pool.tile([L, NG], F32)          # per-group partial sums
        nc.gpsimd.memset(selbuf[:], 0.0)

        def do_dma(q: int):
            for c in range(3):
                src = AP(
                    tensor=embt,
                    offset=(c * 128) * NTOK + q * QW,
                    ap=[[NTOK, 128], [1, QW]],
                )
                nc.sync.dma_start(out=ev[:, c, q * QW : (q + 1) * QW], in_=src)

        def ce_group(g: int):
            # ---- logitsT [17, 512] ----
            lg_ps = ps_l.tile([L, 512], F32, tag="lg_ps")
            for c in range(3):
                nc.tensor.matmul(
                    lg_ps[:],
                    wt_t[:, c * L : (c + 1) * L],
                    ev[:, c, g * 512 : (g + 1) * 512],
                    start=(c == 0),
                    stop=(c == 2),
                )

            # ---- exp(logit + b) -> bf16 ----
            expT = expt_pool.tile([L, 512], BF16, tag="expT")
            nc.scalar.activation(expT[:], lg_ps[:], AF.Exp, bias=bcol_t[:, 0:1], scale=1.0)

            # ---- sumexp row-placement matmul ----
            nc.tensor.matmul(
                sumexp_ps[:],
                selg_t[:, g * NG : (g + 1) * NG],
                expT[:],
                start=(g == 0),
                stop=(g == NG - 1),
            )

            # ---- sel: junk = logit * woh; ScE accumulates into selbuf ----
            junk17 = junk_pool.tile([L, 512], F32, tag="junk17")
            nc.vector.tensor_tensor(
                out=junk17[:],
                in0=lg_ps[:],
                in1=woh_tile(g),
                op=OP.mult,
            )
            junk17b = junk_pool.tile([L, 512], F32, tag="junk17b")
            nc.scalar.activation(
                junk17b[:], junk17[:], AF.Copy,
                accum_out=selbuf[:, g : g + 1],
            )

        woh_sb = consts.tile([L, NTOK], BF16, tag="woh_sb")
        nc.sync.dma_start(out=woh_sb[:], in_=woh.ap())

        def woh_tile(g: int):
            return woh_sb[:, g * 512 : (g + 1) * 512]

        def ctx_quarter(q: int):
            # pairs for columns [q*QW, (q+1)*QW); last quarter drops the
            # final (nonexistent) pair via pairw = 0 and an in-bounds read
            lo = q * QW
            wid = QW if q < NQ - 1 else QW - 1
            dT = sqd_pool.tile([128, 3 * QW], BF16, tag="dT")
            dv = dT[:, :].rearrange("p (c k) -> p c k", k=QW)
            nc.vector.tensor_tensor(
                out=dv[:, :, 0:wid],
                in0=ev[:, :, lo + 1 : lo + 1 + wid],
                in1=ev[:, :, lo : lo + wid],
                op=OP.subtract,
            )
            if wid < QW:
                nc.gpsimd.memset(dv[:, 0:3, wid:QW], 0.0)
            sq = sqd_pool.tile([128, 3 * QW], BF16, tag="sqdT")
            sv = sq[:, :].rearrange("p (c k) -> p c k", k=QW)
            nc.vector.tensor_tensor(out=sv[:, :, :], in0=dv[:, :, :], in1=dv[:, :, :], op=OP.mult)
            for j in range(4):
                g = 4 * q + j
                for c in range(3):
                    nc.tensor.matmul(
                        ctx_ps[:],
                        oneg_t[:, g * NG : (g + 1) * NG],
                        sv[:, c, j * 512 : (j + 1) * 512],
                        start=(g == 0 and c == 0),
                        stop=(g == NG - 1 and c == 2),
                    )

        for q in range(NQ):
            do_dma(q)
            if q > 0:
                ctx_quarter(q - 1)
            for j in range(4):
                ce_group(4 * q + j)
        ctx_quarter(NQ - 1)

        # ---- final reduction ----
        lnsum = junk_pool.tile([NG, 512], F32, tag="lnsum")
        nc.scalar.activation(lnsum[:], sumexp_ps[:], AF.Ln)
        accA = acc_pool.tile([NG, 1], F32)
        junkA = junk_pool.tile([NG, 512], F32, tag="junkA")
        nc.vector.tensor_tensor(out=junkA[:], in0=lnsum[:], in1=cewg_t[:], op=OP.mult)
        junkA2 = junk_pool.tile([NG, 512], F32, tag="junkA2")
        nc.vector.tensor_scalar(
            out=junkA2[:], in0=junkA[:], scalar1=1.0, scalar2=None,
            op0=OP.mult, op1=OP.add, accum_out=accA[:, 0:1],
        )
        selacc = acc_pool.tile([L, 1], F32)
        junkS = junk_pool.tile([L, NG], F32, tag="junkS")
        nc.vector.tensor_scalar(
            out=junkS[:], in0=selbuf[:], scalar1=1.0, scalar2=None,
            op0=OP.mult, op1=OP.add, accum_out=selacc[:, 0:1],
        )
        fin1 = ps_l.tile([1, 1], F32, tag="lg_ps")
        nc.tensor.matmul(fin1[:], accA[:], ones_t[0:NG, :], start=True, stop=True)
        fin3 = ps_l.tile([1, 1], F32, tag="lg_ps")
        nc.tensor.matmul(fin3[:], selacc[:], ones_t[0:L, :], start=True, stop=True)

        ctxacc = acc_pool.tile([NG, 1], F32)
        junkC = junk_pool.tile([NG, 512], F32, tag="junkC")
        nc.vector.tensor_tensor(
            out=junkC[:], in0=ctx_ps[:], in1=pairw_t[:], op=OP.mult,
        )
        junkC2 = junk_pool.tile([NG, 512], F32, tag="junkC2")
        nc.vector.tensor_scalar(
            out=junkC2[:], in0=junkC[:], scalar1=1.0, scalar2=None,
            op0=OP.mult, op1=OP.add, accum_out=ctxacc[:, 0:1],
        )
        fin2 = ps_l.tile([1, 1], F32, tag="lg_ps")
        nc.tensor.matmul(fin2[:], ctxacc[:], ones_t[0:NG, :], start=True, stop=True)

        outs = acc_pool.tile([1, 8], F32)
        nc.vector.memset(outs[:], 0.0)
        nc.scalar.copy(outs[0:1, 0:1], fin1[:])
        nc.scalar.copy(outs[0:1, 1:2], fin2[:])
        nc.scalar.copy(outs[0:1, 2:3], fin3[:])
        nc.sync.dma_start(out=outv.ap(), in_=outs[:])

    nc.compile()
    return nc


# ---------------------------------------------------------------------------
# host-side preparation


def _host_grids(labf: np.ndarray, mskf: np.ndarray):
    """Per-core grids, natural token order (no tiling overlap).

    Returns (woh [L, NTOK] bf16, cewg [NG, 512] f32, pairw [NG, 512] f32)."""
    valid = labf != IGNORE
    lf = labf.astype(np.int64)

    woh = np.zeros((L, NTOK), np.float32)
    lab_c = np.where(valid, lf, 0)
    woh[lab_c, np.arange(NTOK)] = valid.astype(np.float32)
    cewg = valid.astype(np.float32).reshape(NG, 512)

    pair_ok = np.zeros(NTOK, dtype=bool)
    k = np.arange(NTOK - 1)
    in_batch = (k % S) != (S - 1)
    pair_ok[:-1] = in_batch & (lf[:-1] != IGNORE) & (lf[:-1] == lf[1:]) & (lf[:-1] > 0)
    pairw = pair_ok.astype(np.float32).reshape(NG, 512)
    return woh.astype(BF16_NP), cewg, pairw


def _quad_host(fe: np.ndarray, fl: np.ndarray, fm: np.ndarray) -> np.float32:
    """Mirror of the reference quadruplet loss in numpy float32."""
    N = fe.shape[0]
    idx = np.arange(N, dtype=np.int64)
    BIG = N
    fm_b = fm > 0
    is_ent = fm_b & (fl > 0)
    non_ent = fm_b & (fl == 0)
    d_i = np.min(np.where(non_ent, idx, BIG))
    has_non = bool(non_ent.any())

    a_i = np.zeros(L - 1, np.int64)
    p_i = np.zeros(L - 1, np.int64)
    n_i = np.zeros(L - 1, np.int64)
    ok = np.zeros(L - 1, bool)
    for i, t in enumerate(range(1, L)):
        m = is_ent & (fl == t)
        order = np.sort(np.where(m, idx, BIG))
        a_i[i], p_i[i] = order[0], order[1]
        cnt = int(m.sum())
        other = is_ent & (fl != t)
        n_i[i] = np.min(np.where(other, idx, BIG))
        ok[i] = (cnt >= 2) and bool(other.any()) and has_non

    clip = lambda v: np.clip(v, 0, N - 1)
    A = fe[clip(a_i)]
    P = fe[clip(p_i)]
    Ng = fe[clip(n_i)]
    D = fe[clip(np.array([d_i]))]
    eps = np.float32(1e-6)

    def dist(x, y):
        d = (x - y + eps).astype(np.float32)
        return np.sqrt(np.sum(d * d, axis=-1, dtype=np.float32)).astype(np.float32)

    pd, nd, dd = dist(A, P), dist(A, Ng), dist(A, D)
    ql = np.maximum(pd - nd + np.float32(MARGIN), 0) + np.maximum(
        pd - dd + np.float32(2.0 * MARGIN), 0
    )
    qcnt = int(ok.sum())
    quad = float(np.sum(np.where(ok, ql, 0.0), dtype=np.float64)) / max(qcnt, 1)
    return np.float32(quad if qcnt > 0 else 0.0)


_NC_CACHE = {}


def _get_nc():
    if "nc" not in _NC_CACHE:
        _NC_CACHE["nc"] = _build_nc()
    return _NC_CACHE["nc"]


def build_in_maps(embeddings, classifier_w, classifier_b, labels, attention_mask):
    emb = np.ascontiguousarray(np.asarray(embeddings, dtype=np.float32))
    W = np.asarray(classifier_w, dtype=np.float32)
    b = np.asarray(classifier_b, dtype=np.float32)
    lab_f = np.asarray(labels).reshape(-1).astype(np.int64)
    msk_f = np.asarray(attention_mask).reshape(-1).astype(np.int64)
    N = B * S

    emb_bf = emb.reshape(N, H).astype(BF16_NP)
    wt = np.zeros((128, 3 * L), BF16_NP)
    for c in range(3):
        wt[:, c * L : (c + 1) * L] = W[:, c * 128 : (c + 1) * 128].T.astype(BF16_NP)
    bcol = b.reshape(L, 1).astype(np.float32)

    in_maps = []
    for cidx in range(NCORES):
        sl = slice(cidx * NTOK, (cidx + 1) * NTOK)
        woh, cewg, pairw = _host_grids(lab_f[sl], msk_f[sl])
        in_maps.append(
            {
                "embt": np.ascontiguousarray(emb_bf[sl].T),
                "woh": woh,
                "cewg": cewg,
                "pairw": pairw,
                "wt": wt,
                "bcol": bcol,
            }
        )
    return in_maps, emb, lab_f, msk_f, b


def kernel(embeddings, classifier_w, classifier_b, labels, attention_mask):
    from concourse.bass_utils import run_bass_kernel_spmd

    in_maps, emb, lab_f, msk_f, b = build_in_maps(
        embeddings, classifier_w, classifier_b, labels, attention_mask
    )
    N = B * S

    nc = _get_nc()
    res = run_bass_kernel_spmd(nc, in_maps, list(range(NCORES)))

    ce_sum = 0.0
    ctx_sum = 0.0
    for cidx in range(NCORES):
        out = res.results[cidx]["outv"]
        ce_sum += float(out[0, 0]) - float(out[0, 2])
        ctx_sum += float(out[0, 1])

    valid = lab_f != IGNORE
    ce_cnt = int(valid.sum())
    # device sel used logits without bias; correct with sum(w * b[label])
    lab_safe = np.where(valid, lab_f, 0)
    ce_sum -= float(np.sum(np.where(valid, b[lab_safe], 0.0), dtype=np.float64))
    ce = ce_sum / max(ce_cnt, 1)

    pair_ok = np.zeros(N, dtype=bool)
    k = np.arange(N - 1)
    in_batch = (k % S) != (S - 1)
    pair_ok[:-1] = (
        in_batch & (lab_f[:-1] != IGNORE) & (lab_f[:-1] == lab_f[1:]) & (lab_f[:-1] > 0)
    )
    pc = int(pair_ok.sum())
    ctx = (ctx_sum / H) / max(pc, 1) if pc > 0 else 0.0

    quad = _quad_host(emb.reshape(N, H), lab_f, msk_f)

    loss = ce + 0.5 * float(quad) + 0.1 * ctx
    return np.float32(loss)


# revision 10
# speedup vs baseline: 2.9261x; 1.1435x over previous
"""Trainium2 Bass kernel for nn_NERModel loss (CE + quadruplet + context MSE).

v4 strategy (8 NeuronCores, data-parallel over batch):
  - Host pre-transposes each core's embeddings to bf16 embT [384, 8192]
    (h-major): no on-device transposes, no PSUM round-trips, and the DMA
    moves 6.3 MB/core as 16 KB-contiguous descriptors at full rate.
  - Tokens stream in 4 quarters of 2048 columns; 16 CE groups of 512.
  - PE (all bf16): logitsT[17,512] per group over 3 K-chunks; per-token
    sumexp via a row-placement matmul into one persistent PSUM bank;
    ctx per-pair sums via ones-column matmuls into a second bank.
  - DVE: adjacent-column diffs + squares per quarter slab; sel partial
    (logit * one-hot weight) per group, accumulated on ScE.
  - Device returns CE-lse / sel / ctx partials; host adds the tiny
    quadruplet term (49 gathered rows) and combines.
"""

import sys

for _p in ("/opt/trn_rl_repo", "/root/.axon_site/_ro/trn_rl_repo"):
    if _p not in sys.path:
        sys.path.append(_p)

import numpy as np
from contextlib import ExitStack

import ml_dtypes

import concourse.bass as bass
import concourse.bacc as bacc
import concourse.mybir as mybir
from concourse import tile
from concourse.ap import AP

NUM_LABELS = 17
MARGIN = 1.0
IGNORE = -100

B, S, H, L = 64, 1024, 384, NUM_LABELS
NCORES = 8
BP = B // NCORES            # batches per core
NTOK = BP * S               # tokens per core (8192)
NG = 16                     # CE groups of 512 tokens
NQ = 4                      # DMA quarters of 2048 columns
QW = NTOK // NQ             # 2048
F32 = mybir.dt.float32
BF16 = mybir.dt.bfloat16
BF16_NP = ml_dtypes.bfloat16


def _build_nc() -> bass.Bass:
    nc = bacc.Bacc("TRN2", debug=False)

    embt = nc.declare_dram_parameter("embt", [H, NTOK], BF16, isOutput=False)
    woh = nc.declare_dram_parameter("woh", [L, NTOK], BF16, isOutput=False)
    cewg = nc.declare_dram_parameter("cewg", [NG, 512], F32, isOutput=False)
    pairw = nc.declare_dram_parameter("pairw", [NG, 512], F32, isOutput=False)
    wt = nc.declare_dram_parameter("wt", [128, 3 * L], BF16, isOutput=False)
    bcol = nc.declare_dram_parameter("bcol", [L, 1], F32, isOutput=False)
    outv = nc.declare_dram_parameter("outv", [1, 8], F32, isOutput=True)

    AF = mybir.ActivationFunctionType
    AX = mybir.AxisListType
    OP = mybir.AluOpType

    with tile.TileContext(nc) as tc, ExitStack() as ctx:
        consts = ctx.enter_context(tc.tile_pool(name="consts", bufs=1))
        big = ctx.enter_context(tc.tile_pool(name="big", bufs=1))
        sqd_pool = ctx.enter_context(tc.tile_pool(name="sqd", bufs=2))
        expt_pool = ctx.enter_context(tc.tile_pool(name="expt", bufs=2))
        junk_pool = ctx.enter_context(tc.tile_pool(name="junk", bufs=3))
        acc_pool = ctx.enter_context(tc.tile_pool(name="acc", bufs=1))
        ps_l = ctx.enter_context(tc.tile_pool(name="ps_l", bufs=3, space="PSUM"))
        ps_s = ctx.enter_context(tc.tile_pool(name="ps_s", bufs=1, space="PSUM"))
        ps_c = ctx.enter_context(tc.tile_pool(name="ps_c", bufs=1, space="PSUM"))

        def cload(handle, shape, dt):
            t = consts.tile(list(shape), dt, tag=handle.name + "_c")
            nc.sync.dma_start(out=t[:], in_=handle.ap())
            return t

        wt_t = cload(wt, (128, 3 * L), BF16)
        bcol_t = cload(bcol, (L, 1), F32)
        cewg_t = cload(cewg, (NG, 512), F32)
        pairw_t = cload(pairw, (NG, 512), F32)

        # device-built structured consts (DMA of tiny bf16 mats lowers to
        # per-element descriptors and stalls the sync queue for ~20us)
        selg_t = consts.tile([L, NG * NG], BF16, tag="selg")
        oneg_t = consts.tile([128, NG * NG], BF16, tag="oneg")
        ones_t = consts.tile([128, 1], F32, tag="ones")
        nc.gpsimd.memset(selg_t[:], 0.0)
        nc.gpsimd.memset(oneg_t[:], 0.0)
        nc.gpsimd.memset(ones_t[:], 1.0)
        for g in range(NG):
            nc.gpsimd.memset(selg_t[:, g * NG + g : g * NG + g + 1], 1.0)
            nc.gpsimd.memset(oneg_t[:, g * NG + g : g * NG + g + 1], 1.0)

        # embT in 4 quarter tiles [128, 3, QW+1]; col QW duplicates the
        # next quarter's first column so ctx diffs stay tile-local
        QP = QW + 1
        qtiles = [
            big.tile([128, 3 * QP], BF16, tag=f"embT{q}", name=f"embT{q}")
            for q in range(NQ)
        ]
        qviews = [t[:, :].rearrange("p (c k) -> p c k", k=QP) for t in qtiles]

        # persistent accumulators
        sumexp_ps = ps_s.tile([NG, 512], F32)         # [group, token-in-group]
        ctx_ps = ps_c.tile([NG, 512], F32)            # [group, pair-in-group]
        selbuf = acc_pool.tile([L, NG], F32)          # per-group partial sums
        nc.gpsimd.memset(selbuf[:], 0.0)

        def do_dma(q: int):
            w = QP if q < NQ - 1 else QW
            for c in range(3):
                src = AP(
                    tensor=embt,
                    offset=(c * 128) * NTOK + q * QW,
                    ap=[[NTOK, 128], [1, w]],
                )
                nc.sync.dma_start(out=qviews[q][:, c, 0:w], in_=src)

        def ce_group(g: int):
            # ---- logitsT [17, 512] ----
            lg_ps = ps_l.tile([L, 512], F32, tag="lg_ps")
            q, j = divmod(g, 4)
            for c in range(3):
                nc.tensor.matmul(
                    lg_ps[:],
                    wt_t[:, c * L : (c + 1) * L],
                    qviews[q][:, c, j * 512 : (j + 1) * 512],
                    start=(c == 0),
                    stop=(c == 2),
                )

            # ---- exp(logit + b) -> bf16 ----
            expT = expt_pool.tile([L, 512], BF16, tag="expT")
            nc.scalar.activation(expT[:], lg_ps[:], AF.Exp, bias=bcol_t[:, 0:1], scale=1.0)

            # ---- sumexp row-placement matmul ----
            nc.tensor.matmul(
                sumexp_ps[:],
                selg_t[:, g * NG : (g + 1) * NG],
                expT[:],
                start=(g == 0),
                stop=(g == NG - 1),
            )

            # ---- sel: junk = logit * woh; ScE accumulates into selbuf ----
            junk17 = junk_pool.tile([L, 512], F32, tag="junk17")
            nc.vector.tensor_tensor(
                out=junk17[:],
                in0=lg_ps[:],
                in1=woh_tile(g),
                op=OP.mult,
            )
            junk17b = junk_pool.tile([L, 512], F32, tag="junk17b")
            nc.scalar.activation(
                junk17b[:], junk17[:], AF.Copy,
                accum_out=selbuf[:, g : g + 1],
            )

        def woh_tile(g: int):
            return woh_sb[:, g * 512 : (g + 1) * 512]

        def ctx_quarter(q: int):
            # pairs for columns [q*QW, (q+1)*QW); last quarter drops the
            # final (nonexistent) pair via pairw = 0 and an in-bounds read
            wid = QW if q < NQ - 1 else QW - 1
            dT = sqd_pool.tile([128, 3 * QW], BF16, tag="dT")
            dv = dT[:, :].rearrange("p (c k) -> p c k", k=QW)
            nc.vector.tensor_tensor(
                out=dv[:, :, 0:wid],
                in0=qviews[q][:, :, 1 : 1 + wid],
                in1=qviews[q][:, :, 0:wid],
                op=OP.subtract,
            )
            if wid < QW:
                nc.gpsimd.memset(dv[:, 0:3, wid:QW], 0.0)
            sq = sqd_pool.tile([128, 3 * QW], BF16, tag="sqdT")
            sv = sq[:, :].rearrange("p (c k) -> p c k", k=QW)
            nc.vector.tensor_tensor(out=sv[:, :, :], in0=dv[:, :, :], in1=dv[:, :, :], op=OP.mult)
            for j in range(4):
                g = 4 * q + j
                for c in range(3):
                    nc.tensor.matmul(
                        ctx_ps[:],
                        oneg_t[:, g * NG : (g + 1) * NG],
                        sv[:, c, j * 512 : (j + 1) * 512],
                        start=(g == 0 and c == 0),
                        stop=(g == NG - 1 and c == 2),
                    )

        do_dma(0)
        woh_sb = consts.tile([L, NTOK], BF16, tag="woh_sb")
        nc.sync.dma_start(out=woh_sb[:], in_=woh.ap())
        for q in range(NQ):
            if q + 1 < NQ:
                do_dma(q + 1)
            for j in range(4):
                ce_group(4 * q + j)
            ctx_quarter(q)

        # ---- final reduction ----
        lnsum = junk_pool.tile([NG, 512], F32, tag="lnsum")
        nc.scalar.activation(lnsum[:], sumexp_ps[:], AF.Ln)
        accA = acc_pool.tile([NG, 1], F32)
        junkA = junk_pool.tile([NG, 512], F32, tag="junkA")
        nc.vector.tensor_tensor(out=junkA[:], in0=lnsum[:], in1=cewg_t[:], op=OP.mult)
        junkA2 = junk_pool.tile([NG, 512], F32, tag="junkA2")
        nc.vector.tensor_scalar(
            out=junkA2[:], in0=junkA[:], scalar1=1.0, scalar2=None,
            op0=OP.mult, op1=OP.add, accum_out=accA[:, 0:1],
        )
        selacc = acc_pool.tile([L, 1], F32)
        junkS = junk_pool.tile([L, NG], F32, tag="junkS")
        nc.vector.tensor_scalar(
            out=junkS[:], in0=selbuf[:], scalar1=1.0, scalar2=None,
            op0=OP.mult, op1=OP.add, accum_out=selacc[:, 0:1],
        )
        fin1 = ps_l.tile([1, 1], F32, tag="lg_ps")
        nc.tensor.matmul(fin1[:], accA[:], ones_t[0:NG, :], start=True, stop=True)
        fin3 = ps_l.tile([1, 1], F32, tag="lg_ps")
        nc.tensor.matmul(fin3[:], selacc[:], ones_t[0:L, :], start=True, stop=True)

        ctxacc = acc_pool.tile([NG, 1], F32)
        junkC = junk_pool.tile([NG, 512], F32, tag="junkC")
        nc.vector.tensor_tensor(
            out=junkC[:], in0=ctx_ps[:], in1=pairw_t[:], op=OP.mult,
        )
        junkC2 = junk_pool.tile([NG, 512], F32, tag="junkC2")
        nc.vector.tensor_scalar(
            out=junkC2[:], in0=junkC[:], scalar1=1.0, scalar2=None,
            op0=OP.mult, op1=OP.add, accum_out=ctxacc[:, 0:1],
        )
        fin2 = ps_l.tile([1, 1], F32, tag="lg_ps")
        nc.tensor.matmul(fin2[:], ctxacc[:], ones_t[0:NG, :], start=True, stop=True)

        outs = acc_pool.tile([1, 8], F32)
        nc.vector.memset(outs[:], 0.0)
        nc.scalar.copy(outs[0:1, 0:1], fin1[:])
        nc.scalar.copy(outs[0:1, 1:2], fin2[:])
        nc.scalar.copy(outs[0:1, 2:3], fin3[:])
        nc.sync.dma_start(out=outv.ap(), in_=outs[:])

    nc.compile()
    return nc


# ---------------------------------------------------------------------------
# host-side preparation


def _host_grids(labf: np.ndarray, mskf: np.ndarray):
    """Per-core grids, natural token order (no tiling overlap).

    Returns (woh [L, NTOK] bf16, cewg [NG, 512] f32, pairw [NG, 512] f32)."""
    valid = labf != IGNORE
    lf = labf.astype(np.int64)

    woh = np.zeros((L, NTOK), np.float32)
    lab_c = np.where(valid, lf, 0)
    woh[lab_c, np.arange(NTOK)] = valid.astype(np.float32)
    cewg = valid.astype(np.float32).reshape(NG, 512)

    pair_ok = np.zeros(NTOK, dtype=bool)
    k = np.arange(NTOK - 1)
    in_batch = (k % S) != (S - 1)
    pair_ok[:-1] = in_batch & (lf[:-1] != IGNORE) & (lf[:-1] == lf[1:]) & (lf[:-1] > 0)
    pairw = pair_ok.astype(np.float32).reshape(NG, 512)
    return woh.astype(BF16_NP), cewg, pairw


def _quad_host(fe: np.ndarray, fl: np.ndarray, fm: np.ndarray) -> np.float32:
    """Mirror of the reference quadruplet loss in numpy float32."""
    N = fe.shape[0]
    idx = np.arange(N, dtype=np.int64)
    BIG = N
    fm_b = fm > 0
    is_ent = fm_b & (fl > 0)
    non_ent = fm_b & (fl == 0)
    d_i = np.min(np.where(non_ent, idx, BIG))
    has_non = bool(non_ent.any())

    a_i = np.zeros(L - 1, np.int64)
    p_i = np.zeros(L - 1, np.int64)
    n_i = np.zeros(L - 1, np.int64)
    ok = np.zeros(L - 1, bool)
    for i, t in enumerate(range(1, L)):
        m = is_ent & (fl == t)
        order = np.sort(np.where(m, idx, BIG))
        a_i[i], p_i[i] = order[0], order[1]
        cnt = int(m.sum())
        other = is_ent & (fl != t)
        n_i[i] = np.min(np.where(other, idx, BIG))
        ok[i] = (cnt >= 2) and bool(other.any()) and has_non

    clip = lambda v: np.clip(v, 0, N - 1)
    A = fe[clip(a_i)]
    P = fe[clip(p_i)]
    Ng = fe[clip(n_i)]
    D = fe[clip(np.array([d_i]))]
    eps = np.float32(1e-6)

    def dist(x, y):
        d = (x - y + eps).astype(np.float32)
        return np.sqrt(np.sum(d * d, axis=-1, dtype=np.float32)).astype(np.float32)

    pd, nd, dd = dist(A, P), dist(A, Ng), dist(A, D)
    ql = np.maximum(pd - nd + np.float32(MARGIN), 0) + np.maximum(
        pd - dd + np.float32(2.0 * MARGIN), 0
    )
    qcnt = int(ok.sum())
    quad = float(np.sum(np.where(ok, ql, 0.0), dtype=np.float64)) / max(qcnt, 1)
    return np.float32(quad if qcnt > 0 else 0.0)


_NC_CACHE = {}


def _get_nc():
    if "nc" not in _NC_CACHE:
        _NC_CACHE["nc"] = _build_nc()
    return _NC_CACHE["nc"]


def build_in_maps(embeddings, classifier_w, classifier_b, labels, attention_mask):
    emb = np.ascontiguousarray(np.asarray(embeddings, dtype=np.float32))
    W = np.asarray(classifier_w, dtype=np.float32)
    b = np.asarray(classifier_b, dtype=np.float32)
    lab_f = np.asarray(labels).reshape(-1).astype(np.int64)
    msk_f = np.asarray(attention_mask).reshape(-1).astype(np.int64)
    N = B * S

    emb_bf = emb.reshape(N, H).astype(BF16_NP)
    wt = np.zeros((128, 3 * L), BF16_NP)
    for c in range(3):
        wt[:, c * L : (c + 1) * L] = W[:, c * 128 : (c + 1) * 128].T.astype(BF16_NP)
    bcol = b.reshape(L, 1).astype(np.float32)

    in_maps = []
    for cidx in range(NCORES):
        sl = slice(cidx * NTOK, (cidx + 1) * NTOK)
        woh, cewg, pairw = _host_grids(lab_f[sl], msk_f[sl])
        in_maps.append(
            {
                "embt": np.ascontiguousarray(emb_bf[sl].T),
                "woh": woh,
                "cewg": cewg,
                "pairw": pairw,
                "wt": wt,
                "bcol": bcol,
            }
        )
    return in_maps, emb, lab_f, msk_f, b


def kernel(embeddings, classifier_w, classifier_b, labels, attention_mask):
    from concourse.bass_utils import run_bass_kernel_spmd

    in_maps, emb, lab_f, msk_f, b = build_in_maps(
        embeddings, classifier_w, classifier_b, labels, attention_mask
    )
    N = B * S

    nc = _get_nc()
    res = run_bass_kernel_spmd(nc, in_maps, list(range(NCORES)))

    ce_sum = 0.0
    ctx_sum = 0.0
    for cidx in range(NCORES):
        out = res.results[cidx]["outv"]
        ce_sum += float(out[0, 0]) - float(out[0, 2])
        ctx_sum += float(out[0, 1])

    valid = lab_f != IGNORE
    ce_cnt = int(valid.sum())
    # device sel used logits without bias; correct with sum(w * b[label])
    lab_safe = np.where(valid, lab_f, 0)
    ce_sum -= float(np.sum(np.where(valid, b[lab_safe], 0.0), dtype=np.float64))
    ce = ce_sum / max(ce_cnt, 1)

    pair_ok = np.zeros(N, dtype=bool)
    k = np.arange(N - 1)
    in_batch = (k % S) != (S - 1)
    pair_ok[:-1] = (
        in_batch & (lab_f[:-1] != IGNORE) & (lab_f[:-1] == lab_f[1:]) & (lab_f[:-1] > 0)
    )
    pc = int(pair_ok.sum())
    ctx = (ctx_sum / H) / max(pc, 1) if pc > 0 else 0.0

    quad = _quad_host(emb.reshape(N, H), lab_f, msk_f)

    loss = ce + 0.5 * float(quad) + 0.1 * ctx
    return np.float32(loss)
